# revision 6
# baseline (speedup 1.0000x reference)
"""Trainium2 Bass kernel for nn_ConvEmbedder.

out[b, p, e] = (patch(b, p) . conv_w + conv_b) * lin_w[e] + lin_b[e]

data [64, 512, 512] f32, non-overlapping 16x16 patches (1024 per image),
E = 768.  Pure data-parallel over the batch: 8 images per NeuronCore.

Per-core pipeline (per image):
  1. DMA image -> SBUF [128, 4, 512]   (row-group g holds image rows 128g..128g+127)
  2. DVE: tmp = d * wexp               (wexp[p, g, w] = conv_w[p%16, w%16])
  3. PE: 4 accumulating matmuls with block-diagonal ones lhsT -> PSUM t2[32, 512]
       t2[i, 16j+c] = sum_r tmp[row=16*i+r, 16j+c]   (i = absolute patch row)
  4. DVE reduce over c-groups: v[32, 32],  v[i, j] = conv_val(patch(i, j)) - conv_b
  5. SBUF->SBUF DMA flattens v to lhsT[0, 0:1024] (patch-major); lhsT[1, :] = 1.0
  6. PE per 128-patch block: [1; v].T @ [lin_b + conv_b*lin_w; lin_w] -> PSUM [128, 768]
  7. PSUM->SBUF copy split between DVE and ScalarE
  8. contiguous 768 KB DMA store per 2 blocks
"""

import numpy as np

import concourse.bacc as bacc
import concourse.tile as tile
from concourse import mybir
from concourse.bass_utils import run_bass_kernel_spmd

KS = 16          # conv kernel == patch size
E = 768          # embed dim
NCORES = 8
B = 64
H = 512
W = 512
BPC = B // NCORES          # images per core
NH = H // KS               # 32 patch rows (and patch cols) per image
NPATCH = NH * NH           # 1024 patches per image
NG = H // 128              # 4 row-groups per image
NBLK = NPATCH // 128       # 8 expansion blocks of 128 patches
DT = mybir.dt.float32
SPLIT = 352                # DVE copies cols [0:SPLIT), ScalarE [SPLIT:768)

_NC = None
_LAST_RESULTS = None       # BassKernelResults of the last run (for test harness)


def _build_nc():
    nc = bacc.Bacc("TRN2", target_bir_lowering=False, debug=False)
    data_t = nc.dram_tensor("data", [BPC, H, W], DT, kind="ExternalInput")
    wexp_t = nc.dram_tensor("wexp", [128, NG, W], DT, kind="ExternalInput")
    bd_t = nc.dram_tensor("bd", [128, NG, NH], DT, kind="ExternalInput")
    lwb_t = nc.dram_tensor("lwb", [2, E], DT, kind="ExternalInput")
    out_t = nc.dram_tensor("out", [BPC, NPATCH, E], DT, kind="ExternalOutput")

    with tile.TileContext(nc) as tc:
        with (
            tc.tile_pool(name="singles", bufs=1) as singles,
            tc.tile_pool(name="dpool", bufs=2) as dpool,
            tc.tile_pool(name="tpool", bufs=2) as tpool,
            tc.tile_pool(name="vpool", bufs=2) as vpool,
            tc.tile_pool(name="lpool", bufs=2) as lpool,
            tc.tile_pool(name="opool", bufs=4) as opool,
            tc.tile_pool(name="pt2p", bufs=2, space="PSUM") as pt2p,
            tc.tile_pool(name="pop", bufs=3, space="PSUM") as pop,
        ):
            wexp = singles.tile([128, NG, W], DT)
            nc.sync.dma_start(out=wexp[:], in_=wexp_t.ap())
            bd = singles.tile([128, NG, NH], DT)
            nc.sync.dma_start(out=bd[:], in_=bd_t.ap())
            lwb = singles.tile([2, E], DT)
            nc.sync.dma_start(out=lwb[:], in_=lwb_t.ap())

            for img in range(BPC):
                # 1. load image: d[p, g, w] = data[img, 128*g + p, w]
                d = dpool.tile([128, NG, W], DT)
                nc.sync.dma_start(
                    out=d[:],
                    in_=data_t.ap()[img].rearrange("(g p) w -> p g w", p=128),
                )
                # 2. elementwise conv-weight multiply
                tmp = tpool.tile([128, NG, W], DT)
                nc.vector.tensor_mul(tmp[:], d[:], wexp[:])
                # 3. reduce the 16 rows of each patch-row via block-diag ones
                pt2 = pt2p.tile([NH, W], DT)
                for g in range(NG):
                    nc.tensor.matmul(
                        pt2[:],
                        bd[:, g, :],
                        tmp[:, g, :],
                        start=(g == 0),
                        stop=(g == NG - 1),
                    )
                # 4. reduce the 16 cols of each patch
                v8 = vpool.tile([NH, NH], DT)
                nc.vector.tensor_reduce(
                    out=v8[:],
                    in_=pt2[:].rearrange("i (j c) -> i j c", c=KS),
                    axis=mybir.AxisListType.X,
                    op=mybir.AluOpType.add,
                )
                # 5. row 0 = ones, row 1 = v flattened patch-major
                lhsT = lpool.tile([2, NPATCH], DT)
                nc.gpsimd.memset(lhsT[0:1, :], 1.0)
                nc.sync.dma_start(
                    out=lhsT[1:2, :].rearrange("o (i j) -> o i j", j=NH),
                    in_=v8[:],
                )
                # 6-8. expansion: out[p, e] = v[p]*lin_w[e] + lin_b_eff[e]
                for bb in range(0, NBLK, 2):
                    ot = opool.tile([128, 2, E], DT)
                    for sub in range(2):
                        blk = bb + sub
                        lhsT_blk = lhsT[:, 128 * blk:128 * (blk + 1)]
                        po = pop.tile([128, E], DT)
                        nc.tensor.matmul(
                            po[:, 0:512], lhsT_blk, lwb[:, 0:512],
                            start=True, stop=True,
                        )
                        nc.tensor.matmul(
                            po[:, 512:E], lhsT_blk, lwb[:, 512:E],
                            start=True, stop=True,
                        )
                        nc.vector.tensor_copy(ot[:, sub, 0:SPLIT], po[:, 0:SPLIT])
                        nc.scalar.copy(ot[:, sub, SPLIT:E], po[:, SPLIT:E])
                    nc.scalar.dma_start(
                        out=out_t.ap()[img, 128 * bb:128 * (bb + 2), :]
                        .rearrange("(blk p) e -> p blk e", p=128),
                        in_=ot[:],
                    )
    nc.compile()
    return nc


def _get_nc():
    global _NC
    if _NC is None:
        _NC = _build_nc()
    return _NC


def _prepare_in_maps(data, conv_w, conv_b, lin_w, lin_b):
    data = np.ascontiguousarray(np.asarray(data, dtype=np.float32))
    conv_w = np.asarray(conv_w, dtype=np.float32).reshape(KS, KS)
    conv_b = np.float32(np.asarray(conv_b, dtype=np.float32))
    lin_w = np.asarray(lin_w, dtype=np.float32).reshape(E)
    lin_b = np.asarray(lin_b, dtype=np.float32).reshape(E)

    # wexp[p, g, w] = conv_w[p % 16, w % 16]
    wexp = np.ascontiguousarray(
        np.broadcast_to(
            np.tile(conv_w, (128 // KS, W // KS))[:, None, :], (128, NG, W)
        )
    )
    # bd[row, g, m] = 1 iff m == 8*g + row//16  (block-diagonal ones)
    bd = np.zeros((128, NG, NH), dtype=np.float32)
    rows = np.arange(128)
    for g in range(NG):
        bd[rows, g, (128 // KS) * g + rows // KS] = 1.0
    # fold conv_b: v*lin_w + (conv_b*lin_w + lin_b)
    lin_b_eff = (
        np.float64(conv_b) * lin_w.astype(np.float64) + lin_b.astype(np.float64)
    ).astype(np.float32)
    lwb = np.ascontiguousarray(np.stack([lin_b_eff, lin_w], axis=0))

    return [
        {
            "data": np.ascontiguousarray(data[i * BPC:(i + 1) * BPC]),
            "wexp": wexp,
            "bd": bd,
            "lwb": lwb,
        }
        for i in range(NCORES)
    ]


def kernel(data, conv_w, conv_b, lin_w, lin_b):
    global _LAST_RESULTS
    in_maps = _prepare_in_maps(data, conv_w, conv_b, lin_w, lin_b)
    nc = _get_nc()
    res = run_bass_kernel_spmd(nc, in_maps, core_ids=list(range(NCORES)))
    _LAST_RESULTS = res
    return np.concatenate([r["out"] for r in res.results], axis=0)
